# revision 14
# baseline (speedup 1.0000x reference)
"""Multi-head self-attention (B=2, S=2048, E=1024, H=16) on 8 NeuronCores.

Sharding: core c handles batch b = c // 4 and head group g = c % 4 (4 heads).
Each core computes: qkv projection for its heads, attention, and a partial
c_proj (row-slice of W_proj); the host sums the 4 partials per batch.

Device pipeline (all matmuls bf16 inputs, fp32 PSUM accumulation):
  qkT  [2*GW, S]  = (wqk slice).T-stationary @ xT          (Q^T and K^T, head-major)
  V    [S, GW]    = xT-stationary @ wv                     (natural layout, +ones col)
  sT   [kt, q]    = K^T.T @ Q^T  (per head, 64-deep contraction, row-packed pairs)
  expT            = exp(0.125 * sT)  on ACT, bf16 out      (no max-sub: |s/8| <~ 6)
  yT'  [65, q]    = V'.T @ expT  (row 64 = softmax sums via ones column)
  yn   [64, q]    = yT' * (1/sums) broadcast               (per-head normalization)
  out  [S, E]    += yn.T-stationary @ wp                   (partial; host reduces)
"""

import numpy as np
import ml_dtypes

B, S, E, H, D = 2, 2048, 1024, 16, 64
HPC = 4              # heads per core
GW = HPC * D         # 256: per-core width of q/k/v blocks
VW = D + 1           # 65: v columns + ones column
NCORES = 8
ET = E // 128        # 8 contraction tiles for the projections
NQT = S // 512       # 4 query-column tiles
NKT = S // 128       # 16 key tiles
BF16 = ml_dtypes.bfloat16

_CACHE = {}


def _build():
    import concourse.bass as bass
    import concourse.mybir as mybir
    import concourse.tile as tile
    from concourse.tile import add_dep_helper
    from concourse import bacc

    f32, bf16 = mybir.dt.float32, mybir.dt.bfloat16
    Exp = mybir.ActivationFunctionType.Exp

    nc = bacc.Bacc("TRN2", target_bir_lowering=False, debug=False, num_devices=NCORES)
    xT = nc.dram_tensor("xT", [E, S], bf16, kind="ExternalInput").ap()
    wqk = nc.dram_tensor("wqk", [E, 2 * GW], bf16, kind="ExternalInput").ap()
    wv = nc.dram_tensor("wv", [E, GW], bf16, kind="ExternalInput").ap()
    bqk = nc.dram_tensor("bqk", [2 * GW, 1], f32, kind="ExternalInput").ap()
    bv = nc.dram_tensor("bv", [GW], f32, kind="ExternalInput").ap()
    wp = nc.dram_tensor("wp", [GW, E], bf16, kind="ExternalInput").ap()
    out = nc.dram_tensor("out", [S, E], f32, kind="ExternalOutput").ap()

    xT_t = xT.rearrange("(t p) s -> t p s", p=128)
    wqk_t = wqk.rearrange("(t p) n -> t p n", p=128)
    wv_t = wv.rearrange("(t p) n -> t p n", p=128)
    wp_t = wp.rearrange("(t p) n -> t p n", p=128)
    bqk_t = bqk.rearrange("(t p) o -> t p o", p=128)
    out_t = out.rearrange("(t p) n -> t p n", p=128)

    with tile.TileContext(nc) as tc:
        with (
            tc.tile_pool(name="xp", bufs=ET) as xp,
            tc.tile_pool(name="wqkp", bufs=ET) as wqkp,
            tc.tile_pool(name="wvp", bufs=ET) as wvp,
            tc.tile_pool(name="wpp", bufs=2) as wpp,
            tc.tile_pool(name="cst", bufs=1) as cst,
            tc.tile_pool(name="qkp", bufs=4) as qkp,
            tc.tile_pool(name="vp", bufs=NKT) as vp,
            tc.tile_pool(name="ep", bufs=2) as ep,
            tc.tile_pool(name="ynp", bufs=2) as ynp,
            tc.tile_pool(name="rp", bufs=4) as rp,
            tc.tile_pool(name="op", bufs=4) as op,
            tc.tile_pool(name="spool", bufs=2, space="PSUM") as spool,
            tc.tile_pool(name="ppool", bufs=4, space="PSUM") as ppool,
        ):
            # ---- load weights/inputs (interleaved so low-e tiles land first) ----
            xt, wqk_sb, wv_sb = [], [], []
            for e in range(ET):
                t = xp.tile([128, S], bf16, tag="xt", name=f"xt{e}")
                nc.sync.dma_start(t[:], xT_t[e])
                xt.append(t)
                t = wqkp.tile([128, 2 * GW], bf16, tag="wqk", name=f"wqk{e}")
                nc.sync.dma_start(t[:], wqk_t[e])
                wqk_sb.append(t)
                t = wvp.tile([128, GW], bf16, tag="wv", name=f"wv{e}")
                nc.sync.dma_start(t[:], wv_t[e])
                wv_sb.append(t)
            wp_sb = []
            for j in range(2):
                t = wpp.tile([128, E], bf16, tag="wp")
                nc.sync.dma_start(t[:], wp_t[j])
                wp_sb.append(t)
            bqk_sb = []
            for m in range(2 * GW // 128):
                t = cst.tile([128, 1], f32, tag=f"bqk{m}")
                nc.sync.dma_start(t[:], bqk_t[m])
                bqk_sb.append(t)
            bv_bc = cst.tile([128, GW], f32, tag="bv")
            bv_b = bass.AP(tensor=bv.tensor, offset=bv.offset, ap=[[0, 128], *bv.ap])
            nc.sync.dma_start(bv_bc[:], bv_b)

            # ---- qkT: [2*GW, S] = wqk.T @ x.T ----
            qk_sb = []
            for m in range(2 * GW // 128):
                qk_sb.append(qkp.tile([128, S], bf16, tag="qk", name=f"qk{m}"))
            for m in range(2 * GW // 128):
                psA = spool.tile([128, 1024], f32, tag="s")
                psB = spool.tile([128, 1024], f32, tag="s")
                ps = [psA[:, 0:512], psA[:, 512:1024], psB[:, 0:512], psB[:, 512:1024]]
                for e in range(ET):
                    for n in range(4):
                        nc.tensor.matmul(
                            ps[n],
                            wqk_sb[e][:, m * 128 : (m + 1) * 128],
                            xt[e][:, n * 512 : (n + 1) * 512],
                            start=(e == 0),
                            stop=(e == ET - 1),
                        )
                nc.vector.tensor_scalar_add(qk_sb[m][:, 0:1024], psA[:], bqk_sb[m])
                nc.vector.tensor_scalar_add(qk_sb[m][:, 1024:2048], psB[:], bqk_sb[m])

            # ---- V: [S, GW] with ones column per head ----
            v_sb = []
            for mt in range(NKT):
                pv = ppool.tile([128, 512], f32, tag="p")
                for e in range(ET):
                    nc.tensor.matmul(
                        pv[:, 0:GW],
                        xt[e][:, mt * 128 : (mt + 1) * 128],
                        wv_sb[e][:],
                        start=(e == 0),
                        stop=(e == ET - 1),
                    )
                vt = vp.tile([128, HPC * VW], bf16, tag="v")
                vt_h = vt.rearrange("p (h w) -> p h w", w=VW)
                nc.vector.tensor_add(
                    vt_h[:, :, 0:D],
                    pv[:, 0:GW].rearrange("p (h d) -> p h d", d=D),
                    bv_bc.rearrange("p (h d) -> p h d", d=D),
                )
                nc.vector.memset(vt_h[:, :, D : D + 1], 1.0)
                v_sb.append(vt)

            # ---- attention, with c_proj of the previous query tile interleaved
            # into the exp-bound scores loop (keeps the PE stream gap-free) ----
            yn_sb = [ynp.tile([128, S], bf16, tag="yn", name=f"yn{j}") for j in range(2)]

            def emit_proj(mt, nt):
                pp = ppool.tile([128, 512], f32, tag="p", name="pp")
                for j in range(2):
                    nc.tensor.matmul(
                        pp[:],
                        yn_sb[j][:, mt * 128 : (mt + 1) * 128],
                        wp_sb[j][:, nt * 512 : (nt + 1) * 512],
                        start=(j == 0),
                        stop=(j == 1),
                    )
                ot = op.tile([128, 512], f32, tag="o", name="ot")
                nc.vector.tensor_copy(ot[:], pp[:])
                nc.sync.dma_start(out_t[mt][:, nt * 512 : (nt + 1) * 512], ot[:])

            def emit_av(unit, kg):
                pair, eT, pav, _ = unit
                for sub in range(2):
                    kt = 2 * kg + sub
                    for h in range(2):
                        nc.tensor.matmul(
                            pav[h][0:VW, :],
                            v_sb[kt][:, (2 * pair + h) * VW : (2 * pair + h + 1) * VW],
                            eT[h][:, kt * 512 : (kt + 1) * 512],
                            start=(kt == 0),
                            stop=(kt == NKT - 1),
                        )

            def emit_norm(unit):
                pair, _, pav, qs = unit
                for h in range(2):
                    rs = rp.tile([1, 512], f32, tag="rs", name="rs")
                    nc.vector.tensor_copy(rs[:], pav[h][D : D + 1, :])
                    r = rp.tile([1, 512], f32, tag="r", name="r")
                    nc.vector.reciprocal_approx_fast(r[:], rs[:])
                    rb = rp.tile([64, 512], f32, tag="rb", name="rb")
                    nc.gpsimd.partition_broadcast(rb[:], r[:])
                    nc.vector.tensor_mul(
                        yn_sb[pair][64 * h : 64 * (h + 1), qs], pav[h][0:D, :], rb[:]
                    )

            # attn@v lags two kg steps behind scores/exp so the PE always has
            # scores(kg+1) first in its stream when a score slot frees; c_proj
            # of qt-1 drips into pair-1 units at kg>=2 (PSUM slot pressure).
            AV_LAG = 2
            av_queue = []  # (unit, kg); emit_norm(unit) after its kg==last av

            def pop_av():
                u, k = av_queue.pop(0)
                emit_av(u, k)
                if k == NKT // 2 - 1:
                    emit_norm(u)

            for qt in range(NQT):
                qs = slice(qt * 512, (qt + 1) * 512)
                proj_work = (
                    [(mt, nt) for mt in range((qt - 1) * 4, qt * 4) for nt in range(2)]
                    if qt > 0
                    else []
                )
                for pair in range(2):
                    qT = qk_sb[pair]          # Q^T of heads (2*pair, 2*pair+1)
                    kT = qk_sb[2 + pair]      # K^T of same heads
                    eT = [
                        ep.tile([128, NKT * 512], bf16, tag=f"e{h}", name=f"eT{h}")
                        for h in range(2)
                    ]
                    pav = [
                        ppool.tile([128, 512], f32, tag="p", name=f"pav{h}")
                        for h in range(2)
                    ]
                    unit = (pair, eT, pav, qs)
                    for kg in range(NKT // 2):
                        sl = [
                            spool.tile([128, 1024], f32, tag="s", name=f"sl{i}")
                            for i in range(2)
                        ]
                        prev_mm = None
                        for sub in range(2):  # kt pair
                            kt = 2 * kg + sub
                            for h in range(2):  # row-packed head pair
                                pr = slice(64 * h, 64 * (h + 1))
                                mm = nc.tensor.matmul(
                                    sl[h][:, sub * 512 : (sub + 1) * 512],
                                    kT[pr, kt * 128 : (kt + 1) * 128],
                                    qT[pr, qs],
                                    start=True,
                                    stop=True,
                                )
                                if prev_mm is not None:
                                    # keep the h0/h1 pair adjacent in the PE
                                    # stream so the 64-row tiles overlap
                                    add_dep_helper(
                                        mm.ins, prev_mm.ins, sync=False,
                                        reason="row-pack order",
                                    )
                                prev_mm = mm
                        for h in range(2):
                            nc.scalar.activation(
                                eT[h][:, kg * 1024 : (kg + 1) * 1024],
                                sl[h][:],
                                Exp,
                                scale=1.0 / np.sqrt(D),
                            )
                        av_queue.append((unit, kg))
                        if len(av_queue) > AV_LAG:
                            pop_av()
                        if pair == 1 and kg >= 2 and proj_work:
                            emit_proj(*proj_work.pop(0))
                            if kg >= NKT // 2 - 2 and proj_work:
                                emit_proj(*proj_work.pop(0))
                assert not proj_work
            while av_queue:
                pop_av()
            for mt in range((NQT - 1) * 4, NQT * 4):
                for nt in range(2):
                    emit_proj(mt, nt)

    nc.compile()
    return nc


def _get_nc():
    if "nc" not in _CACHE:
        _CACHE["nc"] = _build()
    return _CACHE["nc"]


def _shard_inputs(x, W_attn, b_attn, W_proj):
    """Per-core input dicts; core c = 4*b + g."""
    in_maps = []
    for c in range(NCORES):
        b, g = divmod(c, 4)
        cs = slice(GW * g, GW * (g + 1))
        xTb = np.ascontiguousarray(x[b].T).astype(BF16)
        wqk = np.concatenate(
            [W_attn[:, cs], W_attn[:, E + GW * g : E + GW * (g + 1)]], axis=1
        ).astype(BF16)
        wv = np.ascontiguousarray(W_attn[:, 2 * E + GW * g : 2 * E + GW * (g + 1)]).astype(BF16)
        bqk = np.concatenate(
            [b_attn[cs], b_attn[E + GW * g : E + GW * (g + 1)]]
        ).astype(np.float32)[:, None]
        bv = np.ascontiguousarray(b_attn[2 * E + GW * g : 2 * E + GW * (g + 1)]).astype(np.float32)
        wpc = np.ascontiguousarray(W_proj[cs, :]).astype(BF16)
        in_maps.append(
            {
                "xT": np.ascontiguousarray(xTb),
                "wqk": np.ascontiguousarray(wqk),
                "wv": wv,
                "bqk": np.ascontiguousarray(bqk),
                "bv": bv,
                "wp": wpc,
            }
        )
    return in_maps


def kernel(x, W_attn, b_attn, W_proj, b_proj, _trace=False):
    from concourse import bass_utils

    x = np.asarray(x, dtype=np.float32)
    W_attn = np.asarray(W_attn, dtype=np.float32)
    b_attn = np.asarray(b_attn, dtype=np.float32)
    W_proj = np.asarray(W_proj, dtype=np.float32)
    b_proj = np.asarray(b_proj, dtype=np.float32)

    nc = _get_nc()
    in_maps = _shard_inputs(x, W_attn, b_attn, W_proj)
    res = bass_utils.run_bass_kernel_spmd(
        nc, in_maps, core_ids=list(range(NCORES)), trace=_trace
    )
    _CACHE["last_result"] = res
    out = np.zeros((B, S, E), dtype=np.float32)
    for c in range(NCORES):
        out[c // 4] += res.results[c]["out"]
    out += b_proj
    return out


# revision 16
# speedup vs baseline: 1.1978x; 1.1978x over previous
"""Multi-head self-attention (B=2, S=2048, E=1024, H=16) on 8 NeuronCores.

Sharding: core c handles batch b = c // 4 and head group g = c % 4 (4 heads).
Each core computes: qkv projection for its heads, attention, and a partial
c_proj (row-slice of W_proj); the host sums the 4 partials per batch.

Device pipeline (all matmuls bf16 inputs, fp32 PSUM accumulation):
  qkT  [2*GW, S]  = (wqk slice).T-stationary @ xT          (Q^T and K^T, head-major)
  V    [S, GW]    = xT-stationary @ wv                     (natural layout, +ones col)
  sT   [kt, q]    = K^T.T @ Q^T  (per head, 64-deep contraction, row-packed pairs)
  expT            = exp(0.125 * sT)  on ACT, bf16 out      (no max-sub: |s/8| <~ 6)
  yT'  [65, q]    = V'.T @ expT  (row 64 = softmax sums via ones column)
  yn   [64, q]    = yT' * (1/sums) broadcast               (per-head normalization)
  out  [S, E]    += yn.T-stationary @ wp                   (partial; host reduces)
"""

import numpy as np
import ml_dtypes

B, S, E, H, D = 2, 2048, 1024, 16, 64
HPC = 4              # heads per core
GW = HPC * D         # 256: per-core width of q/k/v blocks
VW = D + 1           # 65: v columns + ones column
NCORES = 8
ET = E // 128        # 8 contraction tiles for the projections
NQT = S // 512       # 4 query-column tiles
NKT = S // 128       # 16 key tiles
BF16 = ml_dtypes.bfloat16

_CACHE = {}


def _build():
    import concourse.bass as bass
    import concourse.mybir as mybir
    import concourse.tile as tile
    from concourse.tile import add_dep_helper
    from concourse import bacc

    f32, bf16 = mybir.dt.float32, mybir.dt.bfloat16
    Exp = mybir.ActivationFunctionType.Exp

    nc = bacc.Bacc("TRN2", target_bir_lowering=False, debug=False, num_devices=NCORES)
    xT = nc.dram_tensor("xT", [E, S], bf16, kind="ExternalInput").ap()
    wqk = nc.dram_tensor("wqk", [E, 2 * GW], bf16, kind="ExternalInput").ap()
    wv = nc.dram_tensor("wv", [E, GW], bf16, kind="ExternalInput").ap()
    bqk = nc.dram_tensor("bqk", [2 * GW, 1], f32, kind="ExternalInput").ap()
    bv = nc.dram_tensor("bv", [GW], f32, kind="ExternalInput").ap()
    wp = nc.dram_tensor("wp", [GW, E], bf16, kind="ExternalInput").ap()
    out = nc.dram_tensor("out", [S, E], f32, kind="ExternalOutput").ap()

    xT_t = xT.rearrange("(t p) s -> t p s", p=128)
    wqk_t = wqk.rearrange("(t p) n -> t p n", p=128)
    wv_t = wv.rearrange("(t p) n -> t p n", p=128)
    wp_t = wp.rearrange("(t p) n -> t p n", p=128)
    bqk_t = bqk.rearrange("(t p) o -> t p o", p=128)
    out_t = out.rearrange("(t p) n -> t p n", p=128)

    with tile.TileContext(nc) as tc:
        with (
            tc.tile_pool(name="xp", bufs=ET) as xp,
            tc.tile_pool(name="wqkp", bufs=ET) as wqkp,
            tc.tile_pool(name="wvp", bufs=ET) as wvp,
            tc.tile_pool(name="wpp", bufs=2) as wpp,
            tc.tile_pool(name="cst", bufs=1) as cst,
            tc.tile_pool(name="qkp", bufs=4) as qkp,
            tc.tile_pool(name="vp", bufs=NKT) as vp,
            tc.tile_pool(name="ep", bufs=2) as ep,
            tc.tile_pool(name="ynp", bufs=2) as ynp,
            tc.tile_pool(name="rp", bufs=4) as rp,
            tc.tile_pool(name="op", bufs=4) as op,
            tc.tile_pool(name="spool", bufs=2, space="PSUM") as spool,
            tc.tile_pool(name="ppool", bufs=4, space="PSUM") as ppool,
        ):
            # ---- load weights/inputs (interleaved so low-e tiles land first) ----
            xt, wqk_sb, wv_sb = [], [], []
            for e in range(ET):
                t = xp.tile([128, S], bf16, tag="xt", name=f"xt{e}")
                nc.sync.dma_start(t[:], xT_t[e])
                xt.append(t)
                t = wqkp.tile([128, 2 * GW], bf16, tag="wqk", name=f"wqk{e}")
                nc.sync.dma_start(t[:], wqk_t[e])
                wqk_sb.append(t)
                t = wvp.tile([128, GW], bf16, tag="wv", name=f"wv{e}")
                nc.sync.dma_start(t[:], wv_t[e])
                wv_sb.append(t)
            wp_sb = []
            for j in range(2):
                t = wpp.tile([128, E], bf16, tag="wp")
                nc.sync.dma_start(t[:], wp_t[j])
                wp_sb.append(t)
            bqk_sb = []
            for m in range(2 * GW // 128):
                t = cst.tile([128, 1], f32, tag=f"bqk{m}")
                nc.sync.dma_start(t[:], bqk_t[m])
                bqk_sb.append(t)
            bv_bc = cst.tile([128, GW], f32, tag="bv")
            bv_b = bass.AP(tensor=bv.tensor, offset=bv.offset, ap=[[0, 128], *bv.ap])
            nc.sync.dma_start(bv_bc[:], bv_b)

            # ---- qkT: [2*GW, S] = wqk.T @ x.T ----
            qk_sb = []
            for m in range(2 * GW // 128):
                qk_sb.append(qkp.tile([128, S], bf16, tag="qk", name=f"qk{m}"))
            for m in range(2 * GW // 128):
                psA = spool.tile([128, 1024], f32, tag="s")
                psB = spool.tile([128, 1024], f32, tag="s")
                ps = [psA[:, 0:512], psA[:, 512:1024], psB[:, 0:512], psB[:, 512:1024]]
                for e in range(ET):
                    for n in range(4):
                        nc.tensor.matmul(
                            ps[n],
                            wqk_sb[e][:, m * 128 : (m + 1) * 128],
                            xt[e][:, n * 512 : (n + 1) * 512],
                            start=(e == 0),
                            stop=(e == ET - 1),
                        )
                nc.vector.tensor_scalar_add(qk_sb[m][:, 0:1024], psA[:], bqk_sb[m])
                nc.vector.tensor_scalar_add(qk_sb[m][:, 1024:2048], psB[:], bqk_sb[m])

            # ---- V: [S, GW] with ones column per head ----
            v_sb = []
            for mt in range(NKT):
                pv = ppool.tile([128, 512], f32, tag="p")
                for e in range(ET):
                    nc.tensor.matmul(
                        pv[:, 0:GW],
                        xt[e][:, mt * 128 : (mt + 1) * 128],
                        wv_sb[e][:],
                        start=(e == 0),
                        stop=(e == ET - 1),
                    )
                vt = vp.tile([128, HPC * VW], bf16, tag="v")
                vt_h = vt.rearrange("p (h w) -> p h w", w=VW)
                nc.vector.tensor_add(
                    vt_h[:, :, 0:D],
                    pv[:, 0:GW].rearrange("p (h d) -> p h d", d=D),
                    bv_bc.rearrange("p (h d) -> p h d", d=D),
                )
                nc.vector.memset(vt_h[:, :, D : D + 1], 1.0)
                v_sb.append(vt)

            # ---- attention, with c_proj of the previous query tile interleaved
            # into the exp-bound scores loop (keeps the PE stream gap-free) ----
            yn_sb = [ynp.tile([128, S], bf16, tag="yn", name=f"yn{j}") for j in range(2)]

            def emit_proj(mt, nt):
                pp = ppool.tile([128, 512], f32, tag="p", name="pp")
                for j in range(2):
                    nc.tensor.matmul(
                        pp[:],
                        yn_sb[j][:, mt * 128 : (mt + 1) * 128],
                        wp_sb[j][:, nt * 512 : (nt + 1) * 512],
                        start=(j == 0),
                        stop=(j == 1),
                    )
                ot = op.tile([128, 512], f32, tag="o", name="ot")
                nc.vector.tensor_copy(ot[:], pp[:])
                nc.sync.dma_start(out_t[mt][:, nt * 512 : (nt + 1) * 512], ot[:])

            def emit_av(unit, kg):
                pair, eT, pav, _ = unit
                for sub in range(2):
                    kt = 2 * kg + sub
                    for h in range(2):
                        nc.tensor.matmul(
                            pav[h][0:VW, :],
                            v_sb[kt][:, (2 * pair + h) * VW : (2 * pair + h + 1) * VW],
                            eT[:, kt * 1024 + 512 * h : kt * 1024 + 512 * (h + 1)],
                            start=(kt == 0),
                            stop=(kt == NKT - 1),
                        )

            def emit_norm(unit):
                pair, _, pav, qs = unit
                for h in range(2):
                    rs = rp.tile([1, 512], f32, tag="rs", name="rs")
                    nc.vector.tensor_copy(rs[:], pav[h][D : D + 1, :])
                    r = rp.tile([1, 512], f32, tag="r", name="r")
                    nc.vector.reciprocal_approx_fast(r[:], rs[:])
                    rb = rp.tile([64, 512], f32, tag="rb", name="rb")
                    nc.gpsimd.partition_broadcast(rb[:], r[:])
                    nc.vector.tensor_mul(
                        yn_sb[pair][64 * h : 64 * (h + 1), qs], pav[h][0:D, :], rb[:]
                    )

            # attn@v lags two kg steps behind scores/exp so the PE always has
            # scores(kg+1) first in its stream when a score slot frees; c_proj
            # of qt-1 drips into pair-1 units at kg>=2 (PSUM slot pressure).
            AV_LAG = 2
            av_queue = []  # (unit, kg); emit_norm(unit) after its kg==last av

            def pop_av():
                u, k = av_queue.pop(0)
                emit_av(u, k)
                if k == NKT // 2 - 1:
                    emit_norm(u)

            for qt in range(NQT):
                qs = slice(qt * 512, (qt + 1) * 512)
                proj_work = (
                    [(mt, nt) for mt in range((qt - 1) * 4, qt * 4) for nt in range(2)]
                    if qt > 0
                    else []
                )
                for pair in range(2):
                    qT = qk_sb[pair]          # Q^T of heads (2*pair, 2*pair+1)
                    kT = qk_sb[2 + pair]      # K^T of same heads
                    # merged exp tile: chunk kt holds [h0 512 | h1 512]
                    eT = ep.tile([128, NKT * 1024], bf16, tag="e", name="eT")
                    pav = [
                        ppool.tile([128, 512], f32, tag="p", name=f"pav{h}")
                        for h in range(2)
                    ]
                    unit = (pair, eT, pav, qs)
                    for kg in range(NKT // 2):
                        # slot `sub` holds both heads of kt=2*kg+sub, so one
                        # exp covers a head pair and frees the slot the next
                        # step's first score pair needs
                        sl = [
                            spool.tile([128, 1024], f32, tag="s", name=f"sl{i}")
                            for i in range(2)
                        ]
                        prev_mm = None
                        for sub in range(2):  # kt pair
                            kt = 2 * kg + sub
                            for h in range(2):  # row-packed head pair
                                pr = slice(64 * h, 64 * (h + 1))
                                mm = nc.tensor.matmul(
                                    sl[sub][:, h * 512 : (h + 1) * 512],
                                    kT[pr, kt * 128 : (kt + 1) * 128],
                                    qT[pr, qs],
                                    start=True,
                                    stop=True,
                                )
                                if prev_mm is not None:
                                    # keep the h0/h1 pair adjacent in the PE
                                    # stream so the 64-row tiles overlap
                                    add_dep_helper(
                                        mm.ins, prev_mm.ins, sync=False,
                                        reason="row-pack order",
                                    )
                                prev_mm = mm
                        for sub in range(2):
                            kt = 2 * kg + sub
                            nc.scalar.activation(
                                eT[:, kt * 1024 : (kt + 1) * 1024],
                                sl[sub][:],
                                Exp,
                                scale=1.0 / np.sqrt(D),
                            )
                        av_queue.append((unit, kg))
                        if len(av_queue) > AV_LAG:
                            pop_av()
                        if pair == 1 and kg >= 2 and proj_work:
                            emit_proj(*proj_work.pop(0))
                            if kg >= NKT // 2 - 2 and proj_work:
                                emit_proj(*proj_work.pop(0))
                assert not proj_work
            while av_queue:
                pop_av()
            for mt in range((NQT - 1) * 4, NQT * 4):
                for nt in range(2):
                    emit_proj(mt, nt)

    nc.compile()
    return nc


def _get_nc():
    if "nc" not in _CACHE:
        _CACHE["nc"] = _build()
    return _CACHE["nc"]


def _shard_inputs(x, W_attn, b_attn, W_proj):
    """Per-core input dicts; core c = 4*b + g."""
    in_maps = []
    for c in range(NCORES):
        b, g = divmod(c, 4)
        cs = slice(GW * g, GW * (g + 1))
        xTb = np.ascontiguousarray(x[b].T).astype(BF16)
        wqk = np.concatenate(
            [W_attn[:, cs], W_attn[:, E + GW * g : E + GW * (g + 1)]], axis=1
        ).astype(BF16)
        wv = np.ascontiguousarray(W_attn[:, 2 * E + GW * g : 2 * E + GW * (g + 1)]).astype(BF16)
        bqk = np.concatenate(
            [b_attn[cs], b_attn[E + GW * g : E + GW * (g + 1)]]
        ).astype(np.float32)[:, None]
        bv = np.ascontiguousarray(b_attn[2 * E + GW * g : 2 * E + GW * (g + 1)]).astype(np.float32)
        wpc = np.ascontiguousarray(W_proj[cs, :]).astype(BF16)
        in_maps.append(
            {
                "xT": np.ascontiguousarray(xTb),
                "wqk": np.ascontiguousarray(wqk),
                "wv": wv,
                "bqk": np.ascontiguousarray(bqk),
                "bv": bv,
                "wp": wpc,
            }
        )
    return in_maps


def kernel(x, W_attn, b_attn, W_proj, b_proj, _trace=False):
    from concourse import bass_utils

    x = np.asarray(x, dtype=np.float32)
    W_attn = np.asarray(W_attn, dtype=np.float32)
    b_attn = np.asarray(b_attn, dtype=np.float32)
    W_proj = np.asarray(W_proj, dtype=np.float32)
    b_proj = np.asarray(b_proj, dtype=np.float32)

    nc = _get_nc()
    in_maps = _shard_inputs(x, W_attn, b_attn, W_proj)
    res = bass_utils.run_bass_kernel_spmd(
        nc, in_maps, core_ids=list(range(NCORES)), trace=_trace
    )
    _CACHE["last_result"] = res
    out = np.zeros((B, S, E), dtype=np.float32)
    for c in range(NCORES):
        out[c // 4] += res.results[c]["out"]
    out += b_proj
    return out
